# revision 1
# baseline (speedup 1.0000x reference)
"""CTC loss (Keras ctc_batch_cost semantics) on 8 Trainium2 NeuronCores.

Strategy: data-parallel over the batch axis (64 sequences per core). The CTC
forward DP runs in the *linear* probability domain with periodic max-
renormalization (scaled forward algorithm), so each time step is 4 DVE ops on
a [64 batch-partitions, 161 extended-state] tile:

    y = m .* q[s-2]            (skip-transition mask multiply)
    x = q + q[s-1]
    u = x + y
    q' = (u [* 1/z]) .* g_t    (g_t = gathered per-state emission probs)

Emission probs g_t[b,s] = y_pred[b,t,ext[b,s]] are gathered per (b, t-chunk)
by GPSIMD indirect_copy in [t-partition, s-free] layout and transposed to the
chain's [b-partition, (t,s)-free] layout with SBUF->SBUF DMAs. Softmax
normalizers Z[b,t] = sum_c y_pred and the final log-combine are handled by
the scalar engine; the loss is

    loss[b] = sum_t ln Z[b,t] - sum_renorms ln z - ln(qT[S-1] + qT[S-2]).
"""

import functools
import os
import sys

import numpy as np

B, T, C, L = 512, 512, 128, 80
S = 2 * L + 1  # 161
BLANK = C - 1
EPS = 1e-7
NCORES = 8
BPC = B // NCORES  # 64 sequences per core
TC = 64  # time-chunk
NCHUNK = T // TC  # 8
NPAIR = BPC // 2  # 32 pair-tiles (2 sequences each) per chunk
IDXW = 12  # wrapped-index columns, padded even so slices stay 4B-aligned
RENORM = 8  # renormalize every 8 steps
SPAD = S + 2  # zero-padded state row
SG = S + 3  # gather width padded to a multiple of 4 (ISA requirement)


def _emit_kernel(ctx, tc, ypred, idxt, maskt, losst, variant="full"):
    import concourse.bass as bass  # noqa: F401
    import concourse.mybir as mybir

    nc = tc.nc
    f32 = mybir.dt.float32
    Alu = mybir.AluOpType
    Act = mybir.ActivationFunctionType

    singles = ctx.enter_context(tc.tile_pool(name="singles", bufs=1))
    ypool = ctx.enter_context(tc.tile_pool(name="ypool", bufs=2))
    gpool = ctx.enter_context(tc.tile_pool(name="gpool", bufs=2))
    g2pool = ctx.enter_context(tc.tile_pool(name="g2pool", bufs=4))
    zscr = ctx.enter_context(tc.tile_pool(name="zscr", bufs=2))
    small = ctx.enter_context(tc.tile_pool(name="small", bufs=2))
    finp = ctx.enter_context(tc.tile_pool(name="finp", bufs=8))
    psump = ctx.enter_context(tc.tile_pool(name="psum", bufs=1, space="PSUM"))

    # --- constants loaded once -------------------------------------------
    idx_sb = singles.tile([128, NPAIR * IDXW], mybir.dt.uint16)
    nc.sync.dma_start(out=idx_sb[:, :], in_=idxt)
    m_sb = singles.tile([BPC, S], f32)
    nc.sync.dma_start(out=m_sb[:, :], in_=maskt)
    # pre-touch idx on GPSIMD so no gather has to wait for its load DMA
    idx_scr = singles.tile([16, 1], mybir.dt.uint16)
    nc.gpsimd.tensor_copy(out=idx_scr[:, :], in_=idx_sb[0:16, 0:1])

    # Z accumulator: col = chunk*NPAIR + pair, value = sum_c y_pred for the
    # 64 t's x 2 b's living in that pair-tile's partitions.
    zbig = singles.tile([128, NCHUNK * NPAIR], f32)
    # half-selector for the final partition-axis reduction via PE
    halfsel = singles.tile([128, 2], f32)
    nc.vector.memset(halfsel[:, :], 0.0)
    nc.vector.memset(halfsel[0:64, 0:1], 1.0)
    nc.vector.memset(halfsel[64:128, 1:2], 1.0)

    # --- producers: load y chunks, Z row-sums, gathers, b<->t swap -------
    gtiles = []
    for ch in range(NCHUNK):
        t0 = ch * TC
        ytile = ypool.tile([128, NPAIR, C], f32, tag="ychunk")
        # one DMA per (chunk, pair-half): partition p=t, free=(pair,c)
        for h in range(2):
            nc.sync.dma_start(
                out=ytile[64 * h : 64 * h + 64, :, :],
                in_=ypred[h::2, t0 : t0 + TC, :].rearrange("j t c -> t j c"),
            )
        gtile = gpool.tile([BPC, TC * S], f32, tag="gchunk")
        for j in range(NPAIR):
            scr = zscr.tile([128, C], f32, tag="zscratch")
            nc.scalar.activation(
                out=scr[:, :],
                in_=ytile[:, j, :],
                func=Act.Copy,
                bias=EPS,
                accum_out=zbig[:, ch * NPAIR + j : ch * NPAIR + j + 1],
            )
            g2 = g2pool.tile([128, SG], f32, tag="g2")
            # Absorb the gather's sync waits (DMA RAW on ytile, swap-DMA WAR on
            # g2) into a cheap same-engine op: the IndirectCopy ISA struct has
            # too few sync-wait slots for Tile's generated waits.
            nc.gpsimd.tensor_copy(out=g2[0:16, 0:1], in_=ytile[0:16, j, 0:1])
            if variant == "nogather":
                nc.gpsimd.tensor_copy(out=g2[:, :], in_=ytile[:, j, 0:SG])
            else:
                nc.gpsimd.indirect_copy(
                    g2[:, :],
                    ytile[:, j, :],
                    idx_sb[:, j * IDXW : (j + 1) * IDXW],
                    True,
                )
            nc.sync.dma_start(out=gtile[2 * j : 2 * j + 2, :], in_=g2[:, 0:S])
        gtiles.append(gtile)

    # --- the DP chain -----------------------------------------------------
    qa = singles.tile([BPC, SPAD], f32)
    qb = singles.tile([BPC, SPAD], f32)
    xt = singles.tile([BPC, S], f32)
    yt = singles.tile([BPC, S], f32)
    ut = singles.tile([BPC, S], f32)
    nrenorm = (T - 2) // RENORM  # renorms measured at t%8==7, t<511
    zstash = singles.tile([BPC, nrenorm], f32)

    nc.vector.memset(qa[:, :], 0.0)
    nc.vector.memset(qb[:, 0:2], 0.0)
    # q0 = g_0 at s in {0,1}
    nc.vector.tensor_copy(out=qa[:, 2:4], in_=gtiles[0][:, 0:2])

    rz_tiles = {}
    cur, nxt = qa, qb
    nsteps = 1 if variant == "nochain" else T
    for t in range(1, nsteps):
        ch, toff = divmod(t, TC)
        g_slice = gtiles[ch][:, toff * S : (toff + 1) * S]
        nc.vector.tensor_tensor(out=yt[:, :], in0=m_sb[:, :], in1=cur[:, 0:S], op=Alu.mult)
        nc.vector.tensor_tensor(
            out=xt[:, :], in0=cur[:, 2:SPAD], in1=cur[:, 1 : S + 1], op=Alu.add
        )
        nc.vector.tensor_tensor(out=ut[:, :], in0=xt[:, :], in1=yt[:, :], op=Alu.add)
        k, phase = divmod(t, RENORM)
        if variant == "chain_tt":
            nc.vector.tensor_tensor(
                out=nxt[:, 2:SPAD], in0=ut[:, :], in1=g_slice, op=Alu.mult
            )
            if phase == RENORM - 1:
                nc.vector.tensor_scalar(
                    out=nxt[:, 2:SPAD],
                    in0=nxt[:, 2:SPAD],
                    scalar1=1e-10,
                    scalar2=1e10,
                    op0=Alu.max,
                    op1=Alu.min,
                )
        elif phase == RENORM - 1 and k < nrenorm:
            # note: tensor_tensor_reduce would fuse these two, but its ISA
            # encoding fails at runtime on this stack — keep them separate
            nc.vector.tensor_tensor(
                out=nxt[:, 2:SPAD], in0=ut[:, :], in1=g_slice, op=Alu.mult
            )
            nc.vector.reduce_max(
                out=zstash[:, k : k + 1],
                in_=nxt[:, 2:SPAD],
                axis=mybir.AxisListType.X,
            )
            rz = small.tile([BPC, 1], f32, tag="rz")
            nc.vector.reciprocal(out=rz[:, :], in_=zstash[:, k : k + 1])
            rz_tiles[k] = rz
        elif phase == 0 and (t // RENORM - 1) in rz_tiles:
            rz = rz_tiles[t // RENORM - 1]
            nc.vector.scalar_tensor_tensor(
                out=nxt[:, 2:SPAD],
                in0=ut[:, :],
                scalar=rz[:, :],
                in1=g_slice,
                op0=Alu.mult,
                op1=Alu.mult,
            )
        else:
            nc.vector.tensor_tensor(
                out=nxt[:, 2:SPAD], in0=ut[:, :], in1=g_slice, op=Alu.mult
            )
        cur, nxt = nxt, cur

    # --- epilogue: loss = W - r - ln(q[S-1] + q[S-2]) ---------------------
    if variant in ("nochain", "chain_tt"):
        # dummy values so the Ln/reduce epilogue stays finite
        nc.vector.memset(zstash[:, :], 1.0)
        if variant == "nochain":
            nc.vector.memset(cur[:, SPAD - 2 : SPAD], 1.0)
    qsum = finp.tile([BPC, 1], f32, tag="fin")
    nc.vector.tensor_tensor(
        out=qsum[:, :], in0=cur[:, SPAD - 1 : SPAD], in1=cur[:, SPAD - 2 : SPAD - 1], op=Alu.add
    )
    lnq = finp.tile([BPC, 1], f32, tag="fin")
    nc.scalar.activation(out=lnq[:, :], in_=qsum[:, :], func=Act.Ln)
    lnz = finp.tile([BPC, nrenorm], f32, tag="lnz")
    nc.scalar.activation(out=lnz[:, :], in_=zstash[:, :], func=Act.Ln)
    r = finp.tile([BPC, 1], f32, tag="fin")
    nc.vector.reduce_sum(out=r[:, :], in_=lnz[:, :], axis=mybir.AxisListType.X)

    lnZ = singles.tile([128, NCHUNK * NPAIR], f32)
    nc.scalar.activation(out=lnZ[:, :], in_=zbig[:, :], func=Act.Ln)
    wsum = singles.tile([128, NPAIR], f32)
    lnZ_v = lnZ[:, :].rearrange("p (c q) -> p q c", c=NCHUNK)
    nc.vector.reduce_sum(out=wsum[:, :], in_=lnZ_v, axis=mybir.AxisListType.X)
    psw = psump.tile([NPAIR, 2], f32)
    nc.tensor.matmul(psw[:, :], lhsT=wsum[:, :], rhs=halfsel[:, :], start=True, stop=True)
    wpsb = finp.tile([NPAIR, 2], f32, tag="wpsb")
    nc.vector.tensor_copy(out=wpsb[:, :], in_=psw[:, :])
    wb = finp.tile([BPC, 1], f32, tag="fin")
    nc.sync.dma_start(out=wb[:, :], in_=wpsb[:, :])

    t1 = finp.tile([BPC, 1], f32, tag="fin")
    nc.vector.tensor_tensor(out=t1[:, :], in0=wb[:, :], in1=r[:, :], op=Alu.subtract)
    lt = finp.tile([BPC, 1], f32, tag="fin")
    nc.vector.tensor_tensor(out=lt[:, :], in0=t1[:, :], in1=lnq[:, :], op=Alu.subtract)
    nc.sync.dma_start(out=losst, in_=lt[:, :])


@functools.lru_cache(maxsize=4)
def _build(variant="full"):
    from contextlib import ExitStack

    import concourse.bacc as bacc
    import concourse.mybir as mybir
    import concourse.tile as tile

    nc = bacc.Bacc(trn_type="TRN2", target_bir_lowering=False)
    ypred = nc.dram_tensor("y_pred", [BPC, T, C], mybir.dt.float32, kind="ExternalInput")
    idxt = nc.dram_tensor(
        "idx", [128, NPAIR * IDXW], mybir.dt.uint16, kind="ExternalInput"
    )
    maskt = nc.dram_tensor("mask", [BPC, S], mybir.dt.float32, kind="ExternalInput")
    losst = nc.dram_tensor("loss", [BPC, 1], mybir.dt.float32, kind="ExternalOutput")
    with tile.TileContext(nc) as tc:
        with ExitStack() as ctx:
            _emit_kernel(
                ctx, tc, ypred[:, :, :], idxt[:, :], maskt[:, :], losst[:, :], variant
            )
    nc.compile()
    return nc


def _host_prep(y_true):
    """Per-core wrapped gather indices and skip-transition masks."""
    y_true = np.asarray(y_true).astype(np.int64)
    ext = np.full((B, S), BLANK, dtype=np.int64)
    ext[:, 1::2] = y_true
    mask = np.zeros((B, S), dtype=np.float32)
    mask[:, 1] = 1.0
    lab = y_true
    neq = (lab[:, 1:] != lab[:, :-1]).astype(np.float32)
    mask[:, 3::2] = neq

    idx_all = []
    for k in range(NCORES):
        idx = np.zeros((128, NPAIR * IDXW), dtype=np.uint16)
        base = k * BPC
        p = np.arange(128)
        for j in range(NPAIR):
            b = base + 2 * j + (p >= 64).astype(np.int64)
            for f in range(IDXW):
                pos = f * 16 + (p % 16)
                valid = pos < S
                idx[p[valid], j * IDXW + f] = ext[b[valid], pos[valid]]
        idx_all.append(idx)
    return idx_all, mask


def kernel(y_true, y_pred):
    from concourse.bass_utils import run_bass_kernel_spmd

    y_pred = np.ascontiguousarray(np.asarray(y_pred), dtype=np.float32)
    idx_all, mask = _host_prep(y_true)

    nc = _build(os.environ.get("CTC_VARIANT", "full"))
    in_maps = []
    for k in range(NCORES):
        b0 = k * BPC
        in_maps.append(
            {
                "y_pred": np.ascontiguousarray(y_pred[b0 : b0 + BPC]),
                "idx": idx_all[k],
                "mask": np.ascontiguousarray(mask[b0 : b0 + BPC]),
            }
        )
    res = run_bass_kernel_spmd(
        nc,
        in_maps,
        core_ids=list(range(NCORES)),
        trace=bool(int(os.environ.get("CTC_TRACE", "0"))),
    )
    out = np.concatenate([r["loss"] for r in res.results], axis=0)
    if res.exec_time_ns is not None:
        print(f"HW exec time: {res.exec_time_ns} ns", file=sys.stderr)
    return out.astype(np.float32)



# revision 9
# speedup vs baseline: 1.0238x; 1.0238x over previous
"""CTC loss (Keras ctc_batch_cost semantics) on 8 Trainium2 NeuronCores.

v3: forward+backward meet-in-the-middle with "fat" octet gathers.

Chain: each core handles 64 sequences; the DP state tile is [128, 163]:
rows 0-63 run the forward alpha recurrence, rows 64-127 the backward gamma
recurrence in state-REVERSED order, making both the same shifted form:

    Q'[r,s] = G_i[r,s] * (Q[r,s] + tau*Q[r,s-1] + Mt[r,s]*Q[r,s-2])

255 serial macro steps x 4 DVE ops (vs 511 for pure forward). The
exponential state tilt tau^s (tau=0.3 via scalar_tensor_tensor + a
tau^2-scaled mask) keeps both chains' state profiles overlapping in fp32 at
the join; the per-state tilt factors cancel up to the constant tau^-160.

Producers: y is loaded per (seq-octet, 64-time chunk, direction) as
[128, 4*C] tiles where partition 16g+w holds 4 consecutive times of sequence
8o+g (backward chunks block-time-reversed via a negative non-leading DMA
stride; the within-block reversal is folded into gather indices). One GPSIMD
indirect_copy per tile gathers all 64 times x 161 extended states for 8
sequences (indices shared per 16-partition group = per sequence), and one
SBUF->SBUF DMA per tile transposes to the chain layout in 2.5KB packets
(128 per DMA). Renormalization (every 8 steps, by row sum) runs on the
scalar engine off the critical path.

  loss[b] = sum_t ln Z[b,t] - sum_k ln z_f - sum_k ln z_b - ln P + 160 ln tau
"""

import functools
import os
import sys

import numpy as np

B, T, C, L = 512, 512, 128, 80
S = 2 * L + 1  # 161
BLANK = C - 1
EPS = 1e-7
NCORES = 8
BPC = B // NCORES  # 64 sequences per core
TC = 64  # time-chunk per macro chunk
NMC = 4  # macro chunks (255 chain steps)
TSUB = 4  # times per partition in the gather layout
NW = TC // TSUB  # 16 w-slots per sequence
NOCT = BPC // 8  # 8 seq-octets per core
RN = 8  # renormalize every 8 steps
NRENORM = 31
SPAD = S + 2
GW = TSUB * S  # gather output width per partition = 644 (mult of 4)
IDX3W = 42  # wrapped idx columns (ceil(644/16)=41, padded even for 4B-aligned slices)
RIDXW = 12  # epilogue reversal idx columns
SG = S + 3
TAU = 0.3


def _emit_kernel(ctx, tc, ypred, idxt, maskt, losst):
    import concourse.bass as bass  # noqa: F401
    import concourse.mybir as mybir
    from concourse.ap import AP

    nc = tc.nc
    f32 = mybir.dt.float32
    Alu = mybir.AluOpType
    Act = mybir.ActivationFunctionType

    singles = ctx.enter_context(tc.tile_pool(name="singles", bufs=1))
    ypool = ctx.enter_context(tc.tile_pool(name="ypool", bufs=2))
    gpool = ctx.enter_context(tc.tile_pool(name="gpool", bufs=2))
    g2pool = ctx.enter_context(tc.tile_pool(name="g2pool", bufs=3))
    zscr = ctx.enter_context(tc.tile_pool(name="zscr", bufs=2))
    small = ctx.enter_context(tc.tile_pool(name="small", bufs=2))
    finp = ctx.enter_context(tc.tile_pool(name="finp", bufs=8))
    psump = ctx.enter_context(tc.tile_pool(name="psum", bufs=2, space="PSUM"))

    # --- constants loaded once -------------------------------------------
    NIDX = 2 * NOCT * IDX3W + RIDXW
    idx_sb = singles.tile([128, NIDX], mybir.dt.uint16)
    nc.sync.dma_start(out=idx_sb[:, :], in_=idxt)
    m_sb = singles.tile([128, S], f32)
    nc.sync.dma_start(out=m_sb[:, :], in_=maskt)
    idx_scr = singles.tile([16, 1], mybir.dt.uint16)
    nc.gpsimd.tensor_copy(out=idx_scr[:, :], in_=idx_sb[0:16, 0:1])

    # Z accumulator: col = ((mc*2+dir)*NOCT + o)*TSUB + u
    zbig = singles.tile([128, 2 * NMC * NOCT * TSUB], f32)
    # group selector for the partition-axis W reduction via PE (host input)
    gsel = singles.tile([128, 8], f32)
    nc.sync.dma_start(out=gsel[:, :], in_=tc.gselt)

    # --- producers -------------------------------------------------------
    gtiles = []
    for mc in range(NMC):
        gtile = gpool.tile([128, TC * S], f32, tag="gchunk")
        for dirb in range(2):
            for o in range(NOCT):
                yt = ypool.tile([128, TSUB * C], f32, tag=f"y{dirb}")
                if dirb == 0:
                    t0 = mc * TC
                    v = ypred[8 * o : 8 * o + 8, t0 : t0 + TC, :].rearrange(
                        "g (w u) c -> g w u c", u=TSUB
                    )
                    nc.sync.dma_start(out=yt[:, :], in_=v)
                else:
                    # times [448-64mc, 512-64mc) with w-blocks reversed:
                    # partition (g,w) slot u holds y[seq, 508-64mc-4w+u]
                    t0 = (7 - mc) * TC
                    v = ypred[8 * o : 8 * o + 8, t0 : t0 + TC, :].rearrange(
                        "g (w u) c -> g w u c", u=TSUB
                    )
                    vap = [list(p) for p in v.ap]
                    v = AP(
                        v.tensor,
                        v.offset + (NW - 1) * vap[1][0],
                        [vap[0], [-vap[1][0], NW], vap[2], vap[3]],
                    )
                    nc.scalar.dma_start(out=yt[:, :], in_=v)
                for u in range(TSUB):
                    scr = zscr.tile([128, C], f32, tag=f"zscr{dirb}")
                    col = ((mc * 2 + dirb) * NOCT + o) * TSUB + u
                    nc.scalar.activation(
                        out=scr[:, :],
                        in_=yt[:, u * C : (u + 1) * C],
                        func=Act.Copy,
                        bias=EPS,
                        accum_out=zbig[:, col : col + 1],
                    )
                g2 = g2pool.tile([128, GW], f32, tag=f"g2{dirb}")
                nc.gpsimd.tensor_copy(out=g2[0:16, 0:1], in_=yt[0:16, 0:1])
                ib = (dirb * NOCT + o) * IDX3W
                nc.gpsimd.indirect_copy(
                    g2[:, :], yt[:, :], idx_sb[:, ib : ib + IDX3W], True
                )
                # transpose: partition (g,w), free (u,s) -> row (8o+g+64*dir),
                # free (4w+u)*S + s ; one 128-packet DMA, 2576B per packet
                r0 = 64 * dirb + 8 * o
                out_v = gtile[r0 : r0 + 8, :].rearrange(
                    "g (w q) -> g w q", w=NW
                )
                eng = nc.sync if dirb == 0 else nc.scalar
                eng.dma_start(out=out_v, in_=g2[:, :])
        gtiles.append(gtile)

    # --- the DP chain -----------------------------------------------------
    qa = singles.tile([128, SPAD], f32)
    qb = singles.tile([128, SPAD], f32)
    xt = singles.tile([128, S], f32)
    yt_ = singles.tile([128, S], f32)
    ut = singles.tile([128, S], f32)
    zstash = singles.tile([128, NRENORM], f32)

    nc.vector.memset(qa[:, :], 0.0)
    nc.vector.memset(qb[:, 0:2], 0.0)
    nc.vector.tensor_copy(out=qa[:, 2:3], in_=gtiles[0][:, 0:1])
    nc.scalar.activation(
        out=qa[:, 3:4], in_=gtiles[0][:, 1:2], func=Act.Copy, scale=TAU
    )

    rz_tiles = {}
    zjunk = singles.tile([128, S], f32)
    cur, nxt = qa, qb
    for i in range(1, 256):
        mc, toff = divmod(i, TC)
        g_slice = gtiles[mc][:, toff * S : (toff + 1) * S]
        nc.vector.tensor_tensor(
            out=yt_[:, :], in0=m_sb[:, :], in1=cur[:, 0:S], op=Alu.mult
        )
        nc.vector.scalar_tensor_tensor(
            out=xt[:, :],
            in0=cur[:, 1 : S + 1],
            scalar=TAU,
            in1=cur[:, 2:SPAD],
            op0=Alu.mult,
            op1=Alu.add,
        )
        nc.vector.tensor_tensor(out=ut[:, :], in0=xt[:, :], in1=yt_[:, :], op=Alu.add)
        if i % RN == 0 and (i // RN - 1) in rz_tiles:
            rz = rz_tiles.pop(i // RN - 1)
            nc.vector.scalar_tensor_tensor(
                out=nxt[:, 2:SPAD],
                in0=ut[:, :],
                scalar=rz[:, :],
                in1=g_slice,
                op0=Alu.mult,
                op1=Alu.mult,
            )
        else:
            nc.vector.tensor_tensor(
                out=nxt[:, 2:SPAD], in0=ut[:, :], in1=g_slice, op=Alu.mult
            )
        cur, nxt = nxt, cur
        if i % RN == RN - 2 and i < 254:
            k = i // RN
            nc.scalar.activation(
                out=zjunk[:, :],
                in_=cur[:, 2:SPAD],
                func=Act.Copy,
                accum_out=zstash[:, k : k + 1],
            )
            rz = small.tile([128, 1], f32, tag="rz")
            nc.vector.reciprocal(out=rz[:, :], in_=zstash[:, k : k + 1])
            rz_tiles[k] = rz

    # --- epilogue ---------------------------------------------------------
    # beta' step on bottom rows: bt = g~ + tau*g~[-1] + m~t*g~[-2]
    bx = finp.tile([64, S], f32, tag="bx")
    nc.vector.scalar_tensor_tensor(
        out=bx[:, :],
        in0=cur[64:128, 1 : S + 1],
        scalar=TAU,
        in1=cur[64:128, 2:SPAD],
        op0=Alu.mult,
        op1=Alu.add,
    )
    by = finp.tile([64, S], f32, tag="by")
    nc.vector.tensor_tensor(
        out=by[:, :], in0=m_sb[64:128, :], in1=cur[64:128, 0:S], op=Alu.mult
    )
    btfull = singles.tile([128, SG], f32)
    nc.vector.memset(btfull[:, :], 0.0)
    nc.vector.tensor_tensor(
        out=btfull[64:128, 0:S], in0=bx[:, :], in1=by[:, :], op=Alu.add
    )
    btrev = singles.tile([128, SG], f32)
    nc.gpsimd.tensor_copy(out=btrev[0:16, 0:1], in_=btfull[0:16, 0:1])
    jr = 2 * NOCT * IDX3W
    nc.gpsimd.indirect_copy(
        btrev[:, :], btfull[:, :], idx_sb[:, jr : jr + RIDXW], True
    )
    balign = finp.tile([64, S], f32, tag="balign")
    nc.sync.dma_start(out=balign[:, :], in_=btrev[64:128, 0:S])
    pjunk = finp.tile([64, S], f32, tag="pjunk")
    phat = finp.tile([64, 1], f32, tag="fin")
    nc.vector.scalar_tensor_tensor(
        out=pjunk[:, :],
        in0=cur[0:64, 2:SPAD],
        scalar=1.0,
        in1=balign[:, :],
        op0=Alu.mult,
        op1=Alu.mult,
        accum_out=phat[:, :],
    )
    lnp = finp.tile([64, 1], f32, tag="fin")
    nc.scalar.activation(out=lnp[:, :], in_=phat[:, :], func=Act.Ln)

    # renorm scale logs
    lnzt = finp.tile([128, NRENORM], f32, tag="lnzt")
    nc.scalar.activation(out=lnzt[:, :], in_=zstash[:, :], func=Act.Ln)
    rfull = finp.tile([128, 1], f32, tag="rfull")
    nc.vector.reduce_sum(out=rfull[:, :], in_=lnzt[:, :], axis=mybir.AxisListType.X)
    rb = finp.tile([64, 1], f32, tag="fin")
    nc.sync.dma_start(out=rb[:, :], in_=rfull[64:128, :])
    rsum = finp.tile([64, 1], f32, tag="fin")
    nc.vector.tensor_tensor(
        out=rsum[:, :], in0=rfull[0:64, :], in1=rb[:, :], op=Alu.add
    )

    # softmax normalizer W: ln(zbig), reduce (u) then (mcd), PE group-sum,
    # PE transpose to octet-major, scatter into [64,1]
    NMCD = 2 * NMC
    lnZ = singles.tile([128, NMCD * NOCT * TSUB], f32)
    nc.scalar.activation(out=lnZ[:, :], in_=zbig[:, :], func=Act.Ln)
    wt1 = singles.tile([128, NMCD * NOCT], f32)
    lv = lnZ[:, :].rearrange("p (m o u) -> p (m o) u", m=NMCD, o=NOCT)
    nc.vector.reduce_sum(out=wt1[:, :], in_=lv, axis=mybir.AxisListType.X)
    wsum3 = singles.tile([128, NOCT], f32)
    lv2 = wt1[:, :].rearrange("p (m o) -> p o m", m=NMCD)
    nc.vector.reduce_sum(out=wsum3[:, :], in_=lv2, axis=mybir.AxisListType.X)
    psw = psump.tile([8, 8], f32, tag="ps1")
    nc.tensor.matmul(psw[:, :], lhsT=gsel[:, :], rhs=wsum3[:, :], start=True, stop=True)
    wsb = finp.tile([8, 8], f32, tag="wsb")
    nc.vector.tensor_copy(out=wsb[:, :], in_=psw[:, :])
    wb = finp.tile([BPC, 1], f32, tag="fin")
    for o in range(NOCT):
        nc.sync.dma_start(out=wb[8 * o : 8 * o + 8, :], in_=wsb[:, o : o + 1])

    # loss = W - Rsum - lnP + 160*ln(tau)
    t1 = finp.tile([BPC, 1], f32, tag="fin")
    nc.vector.tensor_tensor(out=t1[:, :], in0=wb[:, :], in1=rsum[:, :], op=Alu.subtract)
    t2 = finp.tile([BPC, 1], f32, tag="fin")
    nc.vector.tensor_tensor(out=t2[:, :], in0=t1[:, :], in1=lnp[:, :], op=Alu.subtract)
    lt = finp.tile([BPC, 1], f32, tag="fin")
    nc.vector.tensor_scalar(
        out=lt[:, :],
        in0=t2[:, :],
        scalar1=float(S - 1) * float(np.log(TAU)),
        scalar2=0.0,
        op0=Alu.add,
        op1=Alu.add,
    )
    nc.sync.dma_start(out=losst, in_=lt[:, :])


@functools.lru_cache(maxsize=4)
def _build():
    from contextlib import ExitStack

    import concourse.bacc as bacc
    import concourse.mybir as mybir
    import concourse.tile as tile

    nc = bacc.Bacc(trn_type="TRN2", target_bir_lowering=False)
    ypred = nc.dram_tensor("y_pred", [BPC, T, C], mybir.dt.float32, kind="ExternalInput")
    NIDX = 2 * NOCT * IDX3W + RIDXW
    idxt = nc.dram_tensor("idx", [128, NIDX], mybir.dt.uint16, kind="ExternalInput")
    maskt = nc.dram_tensor("mask", [128, S], mybir.dt.float32, kind="ExternalInput")
    losst = nc.dram_tensor("loss", [BPC, 1], mybir.dt.float32, kind="ExternalOutput")
    gselt = nc.dram_tensor("gsel", [128, 8], mybir.dt.float32, kind="ExternalInput")
    with tile.TileContext(nc) as tc:
        tc.gselt = gselt[:, :]
        with ExitStack() as ctx:
            _emit_kernel(ctx, tc, ypred[:, :, :], idxt[:, :], maskt[:, :], losst[:, :])
    nc.compile()
    return nc


def _host_prep(y_true):
    """Octet gather indices (fwd, bwd, epilogue-reversal) and tau^2-scaled
    skip masks for both chain halves."""
    y_true = np.asarray(y_true).astype(np.int64)
    ext = np.full((B, S), BLANK, dtype=np.int64)
    ext[:, 1::2] = y_true
    m = np.zeros((B, S), dtype=np.float32)
    m[:, 1] = 1.0
    neq = (y_true[:, 1:] != y_true[:, :-1]).astype(np.float32)
    m[:, 3::2] = neq
    mt = np.zeros((B, S), dtype=np.float32)  # m~[s'] = m[162-s']
    sp = np.arange(2, S)
    mt[:, sp] = m[:, 162 - sp]
    t2 = np.float32(TAU * TAU)

    NIDX = 2 * NOCT * IDX3W + RIDXW
    # wrapped position of idx entry [p, f] within its 16-partition group
    p = np.arange(128)
    idx_all = []
    mask_all = []
    for k in range(NCORES):
        base = k * BPC
        idx = np.zeros((128, NIDX), dtype=np.uint16)
        for dirb in range(2):
            for o in range(NOCT):
                ib = (dirb * NOCT + o) * IDX3W
                for f in range(IDX3W):
                    pos = f * 16 + (p % 16)  # position in the 644-list
                    valid = pos < GW
                    u, s = pos // S, pos % S
                    g = p // 16
                    seq = base + 8 * o + g
                    if dirb == 0:
                        val = u[valid] * C + ext[seq[valid], s[valid]]
                    else:
                        # u-slot reversal + state reversal
                        val = (TSUB - 1 - u[valid]) * C + ext[
                            seq[valid], (S - 1) - s[valid]
                        ]
                    col = np.zeros(128, dtype=np.uint16)
                    col[valid] = val
                    idx[:, ib + f] = col
        jr = 2 * NOCT * IDX3W
        for f in range(RIDXW):
            pos = f * 16 + (p % 16)
            valid = pos < S
            col = np.zeros(128, dtype=np.uint16)
            col[valid] = (S - 1) - pos[valid]
            idx[:, jr + f] = col
        idx_all.append(idx)

        mask = np.zeros((128, S), dtype=np.float32)
        mask[0:64] = m[base : base + BPC] * t2
        mask[64:128] = mt[base : base + BPC] * t2
        mask_all.append(mask)
    return idx_all, mask_all


def gsel_host():
    g = np.zeros((128, 8), dtype=np.float32)
    for gg in range(8):
        g[16 * gg : 16 * gg + 16, gg] = 1.0
    return g


def kernel(y_true, y_pred):
    from concourse.bass_utils import run_bass_kernel_spmd

    y_pred = np.ascontiguousarray(np.asarray(y_pred), dtype=np.float32)
    idx_all, mask_all = _host_prep(y_true)

    nc = _build()
    in_maps = []
    for k in range(NCORES):
        b0 = k * BPC
        in_maps.append(
            {
                "y_pred": np.ascontiguousarray(y_pred[b0 : b0 + BPC]),
                "idx": idx_all[k],
                "mask": mask_all[k],
                "gsel": gsel_host(),
            }
        )
    res = run_bass_kernel_spmd(
        nc,
        in_maps,
        core_ids=list(range(NCORES)),
        trace=bool(int(os.environ.get("CTC_TRACE", "0"))),
    )
    out = np.concatenate([r["loss"] for r in res.results], axis=0)
    if res.exec_time_ns is not None:
        print(f"HW exec time: {res.exec_time_ns} ns", file=sys.stderr)
    return out.astype(np.float32)


# revision 11
# speedup vs baseline: 1.1840x; 1.1565x over previous
"""CTC loss (Keras ctc_batch_cost semantics) on 8 Trainium2 NeuronCores.

v3: forward+backward meet-in-the-middle with "fat" octet gathers.

Chain: each core handles 64 sequences; the DP state tile is [128, 163]:
rows 0-63 run the forward alpha recurrence, rows 64-127 the backward gamma
recurrence in state-REVERSED order, making both the same shifted form:

    Q'[r,s] = G_i[r,s] * (Q[r,s] + tau*Q[r,s-1] + Mt[r,s]*Q[r,s-2])

255 serial macro steps x 4 DVE ops (vs 511 for pure forward). The
exponential state tilt tau^s (tau=0.3 via scalar_tensor_tensor + a
tau^2-scaled mask) keeps both chains' state profiles overlapping in fp32 at
the join; the per-state tilt factors cancel up to the constant tau^-160.

Producers: y is loaded per (seq-octet, 64-time chunk, direction) as
[128, 4*C] tiles where partition 16g+w holds 4 consecutive times of sequence
8o+g (backward chunks block-time-reversed via a negative non-leading DMA
stride; the within-block reversal is folded into gather indices). One GPSIMD
indirect_copy per tile gathers all 64 times x 161 extended states for 8
sequences (indices shared per 16-partition group = per sequence), and one
SBUF->SBUF DMA per tile transposes to the chain layout in 2.5KB packets
(128 per DMA). Renormalization (every 8 steps, by row sum) runs on the
scalar engine off the critical path.

  loss[b] = sum_t ln Z[b,t] - sum_k ln z_f - sum_k ln z_b - ln P + 160 ln tau
"""

import functools
import os
import sys

import numpy as np

B, T, C, L = 512, 512, 128, 80
S = 2 * L + 1  # 161
BLANK = C - 1
EPS = 1e-7
NCORES = 8
BPC = B // NCORES  # 64 sequences per core
TC = 64  # time-chunk per macro chunk
NMC = 4  # macro chunks (255 chain steps)
TSUB = 4  # times per partition in the gather layout
NW = TC // TSUB  # 16 w-slots per sequence
NOCT = BPC // 8  # 8 seq-octets per core
RN = 8  # renormalize every 8 steps
NRENORM = 31
SPAD = S + 2
GW = TSUB * S  # gather output width per partition = 644 (mult of 4)
IDX3W = 42  # wrapped idx columns (ceil(644/16)=41, padded even for 4B-aligned slices)
RIDXW = 12  # epilogue reversal idx columns
SG = S + 3
TAU = 0.3


def _emit_kernel(ctx, tc, ypred, idxt, maskt, losst):
    import concourse.bass as bass  # noqa: F401
    import concourse.mybir as mybir
    from concourse.ap import AP

    nc = tc.nc
    f32 = mybir.dt.float32
    Alu = mybir.AluOpType
    Act = mybir.ActivationFunctionType

    singles = ctx.enter_context(tc.tile_pool(name="singles", bufs=1))
    ypool = ctx.enter_context(tc.tile_pool(name="ypool", bufs=2))
    gpool = ctx.enter_context(tc.tile_pool(name="gpool", bufs=2))
    g2pool = ctx.enter_context(tc.tile_pool(name="g2pool", bufs=4))
    zscr = ctx.enter_context(tc.tile_pool(name="zscr", bufs=2))
    small = ctx.enter_context(tc.tile_pool(name="small", bufs=2))
    finp = ctx.enter_context(tc.tile_pool(name="finp", bufs=8))
    psump = ctx.enter_context(tc.tile_pool(name="psum", bufs=2, space="PSUM"))

    # --- constants loaded once -------------------------------------------
    NIDX = 2 * NOCT * IDX3W + RIDXW
    idx_sb = singles.tile([128, NIDX], mybir.dt.uint16)
    nc.sync.dma_start(out=idx_sb[:, :], in_=idxt)
    m_sb = singles.tile([128, S], f32)
    nc.sync.dma_start(out=m_sb[:, :], in_=maskt)
    idx_scr = singles.tile([16, 1], mybir.dt.uint16)
    nc.gpsimd.tensor_copy(out=idx_scr[:, :], in_=idx_sb[0:16, 0:1])

    # Z accumulator: col = ((mc*2+dir)*NOCT + o)*TSUB + u
    zbig = singles.tile([128, 2 * NMC * NOCT * TSUB], f32)
    # group selector for the partition-axis W reduction via PE (host input)
    gsel = singles.tile([128, 8], f32)
    nc.sync.dma_start(out=gsel[:, :], in_=tc.gselt)

    # --- producers -------------------------------------------------------
    # Phase A: trigger every y load upfront (fwd on sync, bwd on scalar) so
    # the DMA engines run them all in parallel; deep ypool makes WAR deps moot.
    ytiles = {}
    for mc in range(NMC):
        for dirb in range(2):
            for o in range(NOCT):
                yt = ypool.tile([128, TSUB * C], f32, tag=f"y{dirb}{o}")
                ytiles[(mc, dirb, o)] = yt
                if dirb == 0:
                    t0 = mc * TC
                    v = ypred[8 * o : 8 * o + 8, t0 : t0 + TC, :].rearrange(
                        "g (w u) c -> g w u c", u=TSUB
                    )
                    nc.sync.dma_start(out=yt[:, :], in_=v)
                else:
                    # times [448-64mc, 512-64mc) with w-blocks reversed:
                    # partition (g,w) slot u holds y[seq, 508-64mc-4w+u]
                    t0 = (7 - mc) * TC
                    v = ypred[8 * o : 8 * o + 8, t0 : t0 + TC, :].rearrange(
                        "g (w u) c -> g w u c", u=TSUB
                    )
                    vap = [list(p) for p in v.ap]
                    v = AP(
                        v.tensor,
                        v.offset + (NW - 1) * vap[1][0],
                        [vap[0], [-vap[1][0], NW], vap[2], vap[3]],
                    )
                    nc.scalar.dma_start(out=yt[:, :], in_=v)
    # Phase B: per chunk: Z-sum activations, gathers, transpose DMAs
    gtiles = []
    for mc in range(NMC):
        gtile = gpool.tile([128, TC * S], f32, tag="gchunk")
        for dirb in range(2):
            for o in range(NOCT):
                yt = ytiles[(mc, dirb, o)]
                for u in range(TSUB):
                    scr = zscr.tile([128, C], f32, tag=f"zscr{dirb}")
                    col = ((mc * 2 + dirb) * NOCT + o) * TSUB + u
                    nc.scalar.activation(
                        out=scr[:, :],
                        in_=yt[:, u * C : (u + 1) * C],
                        func=Act.Copy,
                        bias=EPS,
                        accum_out=zbig[:, col : col + 1],
                    )
                g2 = g2pool.tile([128, GW], f32, tag=f"g2{dirb}")
                nc.gpsimd.tensor_copy(out=g2[0:16, 0:1], in_=yt[0:16, 0:1])
                ib = (dirb * NOCT + o) * IDX3W
                nc.gpsimd.indirect_copy(
                    g2[:, :], yt[:, :], idx_sb[:, ib : ib + IDX3W], True
                )
                # transpose: partition (g,w), free (u,s) -> row (8o+g+64*dir),
                # free (4w+u)*S + s ; one 128-packet DMA, 2576B per packet
                r0 = 64 * dirb + 8 * o
                out_v = gtile[r0 : r0 + 8, :].rearrange(
                    "g (w q) -> g w q", w=NW
                )
                eng = nc.sync if dirb == 0 else nc.scalar
                eng.dma_start(out=out_v, in_=g2[:, :])
        gtiles.append(gtile)

    # --- the DP chain -----------------------------------------------------
    qa = singles.tile([128, SPAD], f32)
    qb = singles.tile([128, SPAD], f32)
    xt = singles.tile([128, S], f32)
    yt_ = singles.tile([128, S], f32)
    ut = singles.tile([128, S], f32)
    zstash = singles.tile([128, NRENORM], f32)

    nc.vector.memset(qa[:, :], 0.0)
    nc.vector.memset(qb[:, 0:2], 0.0)
    nc.vector.tensor_copy(out=qa[:, 2:3], in_=gtiles[0][:, 0:1])
    nc.vector.tensor_scalar(
        out=qa[:, 3:4], in0=gtiles[0][:, 1:2], scalar1=TAU, scalar2=0.0,
        op0=Alu.mult, op1=Alu.add,
    )

    rz_tiles = {}
    cur, nxt = qa, qb
    for i in range(1, 256):
        mc, toff = divmod(i, TC)
        g_slice = gtiles[mc][:, toff * S : (toff + 1) * S]
        nc.vector.tensor_tensor(
            out=yt_[:, :], in0=m_sb[:, :], in1=cur[:, 0:S], op=Alu.mult
        )
        nc.vector.scalar_tensor_tensor(
            out=xt[:, :],
            in0=cur[:, 1 : S + 1],
            scalar=TAU,
            in1=cur[:, 2:SPAD],
            op0=Alu.mult,
            op1=Alu.add,
        )
        nc.vector.tensor_tensor(out=ut[:, :], in0=xt[:, :], in1=yt_[:, :], op=Alu.add)
        if i % RN == 0 and (i // RN - 1) in rz_tiles:
            rz = rz_tiles.pop(i // RN - 1)
            nc.vector.scalar_tensor_tensor(
                out=nxt[:, 2:SPAD],
                in0=ut[:, :],
                scalar=rz[:, :],
                in1=g_slice,
                op0=Alu.mult,
                op1=Alu.mult,
            )
        else:
            nc.vector.tensor_tensor(
                out=nxt[:, 2:SPAD], in0=ut[:, :], in1=g_slice, op=Alu.mult
            )
        cur, nxt = nxt, cur
        if i % RN == RN - 1 and i < 255:
            k = i // RN
            nc.vector.reduce_sum(
                out=zstash[:, k : k + 1], in_=cur[:, 2:SPAD],
                axis=mybir.AxisListType.X,
            )
            rz = small.tile([128, 1], f32, tag="rz")
            nc.vector.reciprocal(out=rz[:, :], in_=zstash[:, k : k + 1])
            rz_tiles[k] = rz

    # --- epilogue ---------------------------------------------------------
    # beta' step on bottom rows: bt = g~ + tau*g~[-1] + m~t*g~[-2]
    bx = finp.tile([64, S], f32, tag="bx")
    nc.vector.scalar_tensor_tensor(
        out=bx[:, :],
        in0=cur[64:128, 1 : S + 1],
        scalar=TAU,
        in1=cur[64:128, 2:SPAD],
        op0=Alu.mult,
        op1=Alu.add,
    )
    by = finp.tile([64, S], f32, tag="by")
    nc.vector.tensor_tensor(
        out=by[:, :], in0=m_sb[64:128, :], in1=cur[64:128, 0:S], op=Alu.mult
    )
    btfull = singles.tile([128, SG], f32)
    nc.vector.memset(btfull[:, :], 0.0)
    nc.vector.tensor_tensor(
        out=btfull[64:128, 0:S], in0=bx[:, :], in1=by[:, :], op=Alu.add
    )
    btrev = singles.tile([128, SG], f32)
    nc.gpsimd.tensor_copy(out=btrev[0:16, 0:1], in_=btfull[0:16, 0:1])
    jr = 2 * NOCT * IDX3W
    nc.gpsimd.indirect_copy(
        btrev[:, :], btfull[:, :], idx_sb[:, jr : jr + RIDXW], True
    )
    balign = finp.tile([64, S], f32, tag="balign")
    nc.sync.dma_start(out=balign[:, :], in_=btrev[64:128, 0:S])
    pjunk = finp.tile([64, S], f32, tag="pjunk")
    phat = finp.tile([64, 1], f32, tag="fin")
    nc.vector.scalar_tensor_tensor(
        out=pjunk[:, :],
        in0=cur[0:64, 2:SPAD],
        scalar=1.0,
        in1=balign[:, :],
        op0=Alu.mult,
        op1=Alu.mult,
        accum_out=phat[:, :],
    )
    lnp = finp.tile([64, 1], f32, tag="fin")
    nc.scalar.activation(out=lnp[:, :], in_=phat[:, :], func=Act.Ln)

    # renorm scale logs
    lnzt = finp.tile([128, NRENORM], f32, tag="lnzt")
    nc.scalar.activation(out=lnzt[:, :], in_=zstash[:, :], func=Act.Ln)
    rfull = finp.tile([128, 1], f32, tag="rfull")
    nc.vector.reduce_sum(out=rfull[:, :], in_=lnzt[:, :], axis=mybir.AxisListType.X)
    rb = finp.tile([64, 1], f32, tag="fin")
    nc.sync.dma_start(out=rb[:, :], in_=rfull[64:128, :])
    rsum = finp.tile([64, 1], f32, tag="fin")
    nc.vector.tensor_tensor(
        out=rsum[:, :], in0=rfull[0:64, :], in1=rb[:, :], op=Alu.add
    )

    # softmax normalizer W: ln(zbig), reduce (u) then (mcd), PE group-sum,
    # PE transpose to octet-major, scatter into [64,1]
    NMCD = 2 * NMC
    lnZ = singles.tile([128, NMCD * NOCT * TSUB], f32)
    nc.scalar.activation(out=lnZ[:, :], in_=zbig[:, :], func=Act.Ln)
    wt1 = singles.tile([128, NMCD * NOCT], f32)
    lv = lnZ[:, :].rearrange("p (m o u) -> p (m o) u", m=NMCD, o=NOCT)
    nc.vector.reduce_sum(out=wt1[:, :], in_=lv, axis=mybir.AxisListType.X)
    wsum3 = singles.tile([128, NOCT], f32)
    lv2 = wt1[:, :].rearrange("p (m o) -> p o m", m=NMCD)
    nc.vector.reduce_sum(out=wsum3[:, :], in_=lv2, axis=mybir.AxisListType.X)
    psw = psump.tile([8, 8], f32, tag="ps1")
    nc.tensor.matmul(psw[:, :], lhsT=gsel[:, :], rhs=wsum3[:, :], start=True, stop=True)
    wsb = finp.tile([8, 8], f32, tag="wsb")
    nc.vector.tensor_copy(out=wsb[:, :], in_=psw[:, :])
    wb = finp.tile([BPC, 1], f32, tag="fin")
    for o in range(NOCT):
        nc.sync.dma_start(out=wb[8 * o : 8 * o + 8, :], in_=wsb[:, o : o + 1])

    # loss = W - Rsum - lnP + 160*ln(tau)
    t1 = finp.tile([BPC, 1], f32, tag="fin")
    nc.vector.tensor_tensor(out=t1[:, :], in0=wb[:, :], in1=rsum[:, :], op=Alu.subtract)
    t2 = finp.tile([BPC, 1], f32, tag="fin")
    nc.vector.tensor_tensor(out=t2[:, :], in0=t1[:, :], in1=lnp[:, :], op=Alu.subtract)
    lt = finp.tile([BPC, 1], f32, tag="fin")
    nc.vector.tensor_scalar(
        out=lt[:, :],
        in0=t2[:, :],
        scalar1=float(S - 1) * float(np.log(TAU)),
        scalar2=0.0,
        op0=Alu.add,
        op1=Alu.add,
    )
    nc.sync.dma_start(out=losst, in_=lt[:, :])


@functools.lru_cache(maxsize=4)
def _build():
    from contextlib import ExitStack

    import concourse.bacc as bacc
    import concourse.mybir as mybir
    import concourse.tile as tile

    nc = bacc.Bacc(trn_type="TRN2", target_bir_lowering=False)
    ypred = nc.dram_tensor("y_pred", [BPC, T, C], mybir.dt.float32, kind="ExternalInput")
    NIDX = 2 * NOCT * IDX3W + RIDXW
    idxt = nc.dram_tensor("idx", [128, NIDX], mybir.dt.uint16, kind="ExternalInput")
    maskt = nc.dram_tensor("mask", [128, S], mybir.dt.float32, kind="ExternalInput")
    losst = nc.dram_tensor("loss", [BPC, 1], mybir.dt.float32, kind="ExternalOutput")
    gselt = nc.dram_tensor("gsel", [128, 8], mybir.dt.float32, kind="ExternalInput")
    with tile.TileContext(nc) as tc:
        tc.gselt = gselt[:, :]
        with ExitStack() as ctx:
            _emit_kernel(ctx, tc, ypred[:, :, :], idxt[:, :], maskt[:, :], losst[:, :])
    nc.compile()
    return nc


def _host_prep(y_true):
    """Octet gather indices (fwd, bwd, epilogue-reversal) and tau^2-scaled
    skip masks for both chain halves."""
    y_true = np.asarray(y_true).astype(np.int64)
    ext = np.full((B, S), BLANK, dtype=np.int64)
    ext[:, 1::2] = y_true
    m = np.zeros((B, S), dtype=np.float32)
    m[:, 1] = 1.0
    neq = (y_true[:, 1:] != y_true[:, :-1]).astype(np.float32)
    m[:, 3::2] = neq
    mt = np.zeros((B, S), dtype=np.float32)  # m~[s'] = m[162-s']
    sp = np.arange(2, S)
    mt[:, sp] = m[:, 162 - sp]
    t2 = np.float32(TAU * TAU)

    NIDX = 2 * NOCT * IDX3W + RIDXW
    # wrapped position of idx entry [p, f] within its 16-partition group
    p = np.arange(128)
    idx_all = []
    mask_all = []
    for k in range(NCORES):
        base = k * BPC
        idx = np.zeros((128, NIDX), dtype=np.uint16)
        for dirb in range(2):
            for o in range(NOCT):
                ib = (dirb * NOCT + o) * IDX3W
                for f in range(IDX3W):
                    pos = f * 16 + (p % 16)  # position in the 644-list
                    valid = pos < GW
                    u, s = pos // S, pos % S
                    g = p // 16
                    seq = base + 8 * o + g
                    if dirb == 0:
                        val = u[valid] * C + ext[seq[valid], s[valid]]
                    else:
                        # u-slot reversal + state reversal
                        val = (TSUB - 1 - u[valid]) * C + ext[
                            seq[valid], (S - 1) - s[valid]
                        ]
                    col = np.zeros(128, dtype=np.uint16)
                    col[valid] = val
                    idx[:, ib + f] = col
        jr = 2 * NOCT * IDX3W
        for f in range(RIDXW):
            pos = f * 16 + (p % 16)
            valid = pos < S
            col = np.zeros(128, dtype=np.uint16)
            col[valid] = (S - 1) - pos[valid]
            idx[:, jr + f] = col
        idx_all.append(idx)

        mask = np.zeros((128, S), dtype=np.float32)
        mask[0:64] = m[base : base + BPC] * t2
        mask[64:128] = mt[base : base + BPC] * t2
        mask_all.append(mask)
    return idx_all, mask_all


def gsel_host():
    g = np.zeros((128, 8), dtype=np.float32)
    for gg in range(8):
        g[16 * gg : 16 * gg + 16, gg] = 1.0
    return g


def kernel(y_true, y_pred):
    from concourse.bass_utils import run_bass_kernel_spmd

    y_pred = np.ascontiguousarray(np.asarray(y_pred), dtype=np.float32)
    idx_all, mask_all = _host_prep(y_true)

    nc = _build()
    in_maps = []
    for k in range(NCORES):
        b0 = k * BPC
        in_maps.append(
            {
                "y_pred": np.ascontiguousarray(y_pred[b0 : b0 + BPC]),
                "idx": idx_all[k],
                "mask": mask_all[k],
                "gsel": gsel_host(),
            }
        )
    res = run_bass_kernel_spmd(
        nc,
        in_maps,
        core_ids=list(range(NCORES)),
        trace=bool(int(os.environ.get("CTC_TRACE", "0"))),
    )
    out = np.concatenate([r["loss"] for r in res.results], axis=0)
    if res.exec_time_ns is not None:
        print(f"HW exec time: {res.exec_time_ns} ns", file=sys.stderr)
    return out.astype(np.float32)


# revision 15
# speedup vs baseline: 1.1899x; 1.0049x over previous
"""CTC loss (Keras ctc_batch_cost semantics) on 8 Trainium2 NeuronCores.

v3: forward+backward meet-in-the-middle with "fat" octet gathers.

Chain: each core handles 64 sequences; the DP state tile is [128, 163]:
rows 0-63 run the forward alpha recurrence, rows 64-127 the backward gamma
recurrence in state-REVERSED order, making both the same shifted form:

    Q'[r,s] = G_i[r,s] * (Q[r,s] + tau*Q[r,s-1] + Mt[r,s]*Q[r,s-2])

255 serial macro steps x 4 DVE ops (vs 511 for pure forward). The
exponential state tilt tau^s (tau=0.3 via scalar_tensor_tensor + a
tau^2-scaled mask) keeps both chains' state profiles overlapping in fp32 at
the join; the per-state tilt factors cancel up to the constant tau^-160.

Producers: y is loaded per (seq-octet, 64-time chunk, direction) as
[128, 4*C] tiles where partition 16g+w holds 4 consecutive times of sequence
8o+g (backward chunks block-time-reversed via a negative non-leading DMA
stride; the within-block reversal is folded into gather indices). One GPSIMD
indirect_copy per tile gathers all 64 times x 161 extended states for 8
sequences (indices shared per 16-partition group = per sequence), and one
SBUF->SBUF DMA per tile transposes to the chain layout in 2.5KB packets
(128 per DMA). Renormalization (every 8 steps, by row sum) runs on the
scalar engine off the critical path.

  loss[b] = sum_t ln Z[b,t] - sum_k ln z_f - sum_k ln z_b - ln P + 160 ln tau
"""

import functools
import os
import sys

import numpy as np

B, T, C, L = 512, 512, 128, 80
S = 2 * L + 1  # 161
BLANK = C - 1
EPS = 1e-7
NCORES = 8
BPC = B // NCORES  # 64 sequences per core
TC = 64  # time-chunk per macro chunk
NMC = 4  # macro chunks (255 chain steps)
TSUB = 4  # times per partition in the gather layout
NW = TC // TSUB  # 16 w-slots per sequence
NOCT = BPC // 8  # 8 seq-octets per core
RN = 8  # renormalize every 8 steps
NRENORM = 31
SPAD = S + 2
GW = TSUB * S  # gather output width per partition = 644 (mult of 4)
IDX3W = 42  # wrapped idx columns (ceil(644/16)=41, padded even for 4B-aligned slices)
RIDXW = 12  # epilogue reversal idx columns
SG = S + 3
TAU = 0.3


def _emit_kernel(ctx, tc, ypred, idxt, maskt, losst):
    import concourse.bass as bass  # noqa: F401
    import concourse.mybir as mybir
    from concourse.ap import AP

    nc = tc.nc
    f32 = mybir.dt.float32
    Alu = mybir.AluOpType
    Act = mybir.ActivationFunctionType

    singles = ctx.enter_context(tc.tile_pool(name="singles", bufs=1))
    ypool = ctx.enter_context(tc.tile_pool(name="ypool", bufs=2))
    gpool = ctx.enter_context(tc.tile_pool(name="gpool", bufs=2))
    g2pool = ctx.enter_context(tc.tile_pool(name="g2pool", bufs=6))
    zscr = ctx.enter_context(tc.tile_pool(name="zscr", bufs=2))
    small = ctx.enter_context(tc.tile_pool(name="small", bufs=2))
    finp = ctx.enter_context(tc.tile_pool(name="finp", bufs=8))
    epi = ctx.enter_context(tc.tile_pool(name="epi", bufs=1))
    psump = ctx.enter_context(tc.tile_pool(name="psum", bufs=2, space="PSUM"))

    # --- constants loaded once -------------------------------------------
    NIDX = 2 * NOCT * IDX3W + RIDXW
    idx_sb = singles.tile([128, NIDX], mybir.dt.uint16)
    nc.sync.dma_start(out=idx_sb[:, :], in_=idxt)
    m_sb = singles.tile([128, S], f32)
    nc.sync.dma_start(out=m_sb[:, :], in_=maskt)
    idx_scr = singles.tile([16, 1], mybir.dt.uint16)
    nc.gpsimd.tensor_copy(out=idx_scr[:, :], in_=idx_sb[0:16, 0:1])

    # Z accumulator: col = ((mc*2+dir)*NOCT + o)*TSUB + u
    zbig = singles.tile([128, 2 * NMC * NOCT * TSUB], f32)
    # group selector for the partition-axis W reduction via PE (host input)
    gsel = singles.tile([128, 8], f32)
    nc.sync.dma_start(out=gsel[:, :], in_=tc.gselt)

    # --- producers -------------------------------------------------------
    # Phase A: per-(chunk, direction, octet) load DMAs (3-dim APs, 2KB
    # packets), all triggered upfront; fwd on sync, bwd on scalar.
    ytbigs = {}
    for mc in range(NMC):
        for dirb in range(2):
            ybig = ypool.tile([128, NOCT, TSUB * C], f32, tag=f"ybig{dirb}")
            ytbigs[(mc, dirb)] = ybig
            for o in range(NOCT):
                if dirb == 0:
                    off = 8 * o * T * C + (mc * TC) * C
                    wstep = TSUB * C
                else:
                    # w-blocks time-reversed: partition (g,w) slot u holds
                    # y[seq, (448-64mc) + 4*(NW-1-w) + u]
                    off = (
                        8 * o * T * C
                        + ((7 - mc) * TC) * C
                        + (NW - 1) * TSUB * C
                    )
                    wstep = -TSUB * C
                src = AP(
                    ypred.tensor,
                    ypred.offset + off,
                    [
                        [T * C, 8],      # g (seq within octet)
                        [wstep, NW],     # w block (reversed for bwd)
                        [1, TSUB * C],   # (u, c) contiguous
                    ],
                )
                eng = nc.sync if dirb == 0 else nc.scalar
                eng.dma_start(out=ybig[:, o, :], in_=src)
    # Phase B: per chunk: Z-sum activations, gathers, transpose DMAs
    gtiles = []
    for mc in range(NMC):
        gtile = gpool.tile([128, TC * S], f32, tag="gchunk")
        for dirb in range(2):
            for o in range(NOCT):
                yt = ytbigs[(mc, dirb)]
                for u in range(TSUB):
                    scr = zscr.tile([128, C], f32, tag=f"zscr{dirb}")
                    col = ((mc * 2 + dirb) * NOCT + o) * TSUB + u
                    nc.scalar.activation(
                        out=scr[:, :],
                        in_=yt[:, o, u * C : (u + 1) * C],
                        func=Act.Copy,
                        bias=EPS,
                        accum_out=zbig[:, col : col + 1],
                    )
                g2 = g2pool.tile([128, GW], f32, tag=f"g2{dirb}")
                nc.gpsimd.tensor_copy(out=g2[0:16, 0:1], in_=yt[0:16, o, 0:1])
                ib = (dirb * NOCT + o) * IDX3W
                nc.gpsimd.indirect_copy(
                    g2[:, :], yt[:, o, :], idx_sb[:, ib : ib + IDX3W], True
                )
                # transpose: partition (g,w), free (u,s) -> row (8o+g+64*dir),
                # free (4w+u)*S + s ; one 128-packet DMA, 2576B per packet
                r0 = 64 * dirb + 8 * o
                out_v = gtile[r0 : r0 + 8, :].rearrange(
                    "g (w q) -> g w q", w=NW
                )
                eng = nc.sync if dirb == 0 else nc.scalar
                eng.dma_start(out=out_v, in_=g2[:, :])
        gtiles.append(gtile)

    # --- the DP chain -----------------------------------------------------
    qa = singles.tile([128, SPAD], f32)
    qb = singles.tile([128, SPAD], f32)
    xt = singles.tile([128, S], f32)
    yt_ = singles.tile([128, S], f32)
    ut = singles.tile([128, S], f32)
    zstash = singles.tile([128, NRENORM], f32)

    nc.vector.memset(qa[:, :], 0.0)
    nc.vector.memset(qb[:, 0:2], 0.0)
    nc.vector.tensor_copy(out=qa[:, 2:3], in_=gtiles[0][:, 0:1])
    nc.vector.tensor_scalar(
        out=qa[:, 3:4], in0=gtiles[0][:, 1:2], scalar1=TAU, scalar2=0.0,
        op0=Alu.mult, op1=Alu.add,
    )

    rz_tiles = {}
    cur, nxt = qa, qb
    for i in range(1, 256):
        mc, toff = divmod(i, TC)
        g_slice = gtiles[mc][:, toff * S : (toff + 1) * S]
        nc.vector.tensor_tensor(
            out=yt_[:, :], in0=m_sb[:, :], in1=cur[:, 0:S], op=Alu.mult
        )
        nc.vector.scalar_tensor_tensor(
            out=xt[:, :],
            in0=cur[:, 1 : S + 1],
            scalar=TAU,
            in1=cur[:, 2:SPAD],
            op0=Alu.mult,
            op1=Alu.add,
        )
        nc.vector.tensor_tensor(out=ut[:, :], in0=xt[:, :], in1=yt_[:, :], op=Alu.add)
        if i % RN == 0 and (i // RN - 1) in rz_tiles:
            rz = rz_tiles.pop(i // RN - 1)
            nc.vector.scalar_tensor_tensor(
                out=nxt[:, 2:SPAD],
                in0=ut[:, :],
                scalar=rz[:, :],
                in1=g_slice,
                op0=Alu.mult,
                op1=Alu.mult,
            )
        else:
            nc.vector.tensor_tensor(
                out=nxt[:, 2:SPAD], in0=ut[:, :], in1=g_slice, op=Alu.mult
            )
        cur, nxt = nxt, cur
        if i % RN == RN - 1 and i < 255:
            k = i // RN
            nc.vector.reduce_sum(
                out=zstash[:, k : k + 1], in_=cur[:, 2:SPAD],
                axis=mybir.AxisListType.X,
            )
            rz = small.tile([128, 1], f32, tag="rz")
            nc.vector.reciprocal(out=rz[:, :], in_=zstash[:, k : k + 1])
            rz_tiles[k] = rz

    # --- epilogue ---------------------------------------------------------
    # beta' step on bottom rows: bt = g~ + tau*g~[-1] + m~t*g~[-2]
    bx = epi.tile([64, S], f32, tag="bx")
    nc.vector.scalar_tensor_tensor(
        out=bx[:, :],
        in0=cur[64:128, 1 : S + 1],
        scalar=TAU,
        in1=cur[64:128, 2:SPAD],
        op0=Alu.mult,
        op1=Alu.add,
    )
    by = epi.tile([64, S], f32, tag="by")
    nc.vector.tensor_tensor(
        out=by[:, :], in0=m_sb[64:128, :], in1=cur[64:128, 0:S], op=Alu.mult
    )
    btfull = singles.tile([128, SG], f32)
    nc.vector.memset(btfull[:, :], 0.0)
    nc.vector.tensor_tensor(
        out=btfull[64:128, 0:S], in0=bx[:, :], in1=by[:, :], op=Alu.add
    )
    btrev = singles.tile([128, SG], f32)
    nc.gpsimd.tensor_copy(out=btrev[0:16, 0:1], in_=btfull[0:16, 0:1])
    jr = 2 * NOCT * IDX3W
    nc.gpsimd.indirect_copy(
        btrev[:, :], btfull[:, :], idx_sb[:, jr : jr + RIDXW], True
    )
    balign = epi.tile([64, S], f32, tag="balign")
    nc.sync.dma_start(out=balign[:, :], in_=btrev[64:128, 0:S])
    pjunk = epi.tile([64, S], f32, tag="pjunk")
    phat = finp.tile([64, 1], f32, tag="fin")
    nc.vector.scalar_tensor_tensor(
        out=pjunk[:, :],
        in0=cur[0:64, 2:SPAD],
        scalar=1.0,
        in1=balign[:, :],
        op0=Alu.mult,
        op1=Alu.mult,
        accum_out=phat[:, :],
    )
    lnp = finp.tile([64, 1], f32, tag="fin")
    nc.scalar.activation(out=lnp[:, :], in_=phat[:, :], func=Act.Ln)

    # renorm scale logs
    lnzt = epi.tile([128, NRENORM], f32, tag="lnzt")
    nc.scalar.activation(out=lnzt[:, :], in_=zstash[:, :], func=Act.Ln)
    rfull = epi.tile([128, 1], f32, tag="rfull")
    nc.vector.reduce_sum(out=rfull[:, :], in_=lnzt[:, :], axis=mybir.AxisListType.X)
    rb = finp.tile([64, 1], f32, tag="fin")
    nc.sync.dma_start(out=rb[:, :], in_=rfull[64:128, :])
    rsum = finp.tile([64, 1], f32, tag="fin")
    nc.vector.tensor_tensor(
        out=rsum[:, :], in0=rfull[0:64, :], in1=rb[:, :], op=Alu.add
    )

    # softmax normalizer W: ln(zbig), reduce (u) then (mcd), PE group-sum,
    # PE transpose to octet-major, scatter into [64,1]
    NMCD = 2 * NMC
    lnZ = singles.tile([128, NMCD * NOCT * TSUB], f32)
    nc.scalar.activation(out=lnZ[:, :], in_=zbig[:, :], func=Act.Ln)
    wt1 = singles.tile([128, NMCD * NOCT], f32)
    lv = lnZ[:, :].rearrange("p (m o u) -> p (m o) u", m=NMCD, o=NOCT)
    nc.vector.reduce_sum(out=wt1[:, :], in_=lv, axis=mybir.AxisListType.X)
    wsum3 = singles.tile([128, NOCT], f32)
    lv2 = wt1[:, :].rearrange("p (m o) -> p o m", m=NMCD)
    nc.vector.reduce_sum(out=wsum3[:, :], in_=lv2, axis=mybir.AxisListType.X)
    psw = psump.tile([8, 8], f32, tag="ps1")
    nc.tensor.matmul(psw[:, :], lhsT=gsel[:, :], rhs=wsum3[:, :], start=True, stop=True)
    wsb = epi.tile([8, 8], f32, tag="wsb")
    nc.vector.tensor_copy(out=wsb[:, :], in_=psw[:, :])
    wb = finp.tile([BPC, 1], f32, tag="fin")
    for o in range(NOCT):
        nc.sync.dma_start(out=wb[8 * o : 8 * o + 8, :], in_=wsb[:, o : o + 1])

    # loss = W - Rsum - lnP + 160*ln(tau)
    t1 = finp.tile([BPC, 1], f32, tag="fin")
    nc.vector.tensor_tensor(out=t1[:, :], in0=wb[:, :], in1=rsum[:, :], op=Alu.subtract)
    t2 = finp.tile([BPC, 1], f32, tag="fin")
    nc.vector.tensor_tensor(out=t2[:, :], in0=t1[:, :], in1=lnp[:, :], op=Alu.subtract)
    lt = finp.tile([BPC, 1], f32, tag="fin")
    nc.vector.tensor_scalar(
        out=lt[:, :],
        in0=t2[:, :],
        scalar1=float(S - 1) * float(np.log(TAU)),
        scalar2=0.0,
        op0=Alu.add,
        op1=Alu.add,
    )
    nc.sync.dma_start(out=losst, in_=lt[:, :])


@functools.lru_cache(maxsize=4)
def _build():
    from contextlib import ExitStack

    import concourse.bacc as bacc
    import concourse.mybir as mybir
    import concourse.tile as tile

    nc = bacc.Bacc(trn_type="TRN2", target_bir_lowering=False)
    ypred = nc.dram_tensor("y_pred", [BPC, T, C], mybir.dt.float32, kind="ExternalInput")
    NIDX = 2 * NOCT * IDX3W + RIDXW
    idxt = nc.dram_tensor("idx", [128, NIDX], mybir.dt.uint16, kind="ExternalInput")
    maskt = nc.dram_tensor("mask", [128, S], mybir.dt.float32, kind="ExternalInput")
    losst = nc.dram_tensor("loss", [BPC, 1], mybir.dt.float32, kind="ExternalOutput")
    gselt = nc.dram_tensor("gsel", [128, 8], mybir.dt.float32, kind="ExternalInput")
    with tile.TileContext(nc) as tc:
        tc.gselt = gselt[:, :]
        with ExitStack() as ctx:
            _emit_kernel(ctx, tc, ypred[:, :, :], idxt[:, :], maskt[:, :], losst[:, :])
    nc.compile()
    return nc


def _host_prep(y_true):
    """Octet gather indices (fwd, bwd, epilogue-reversal) and tau^2-scaled
    skip masks for both chain halves."""
    y_true = np.asarray(y_true).astype(np.int64)
    ext = np.full((B, S), BLANK, dtype=np.int64)
    ext[:, 1::2] = y_true
    m = np.zeros((B, S), dtype=np.float32)
    m[:, 1] = 1.0
    neq = (y_true[:, 1:] != y_true[:, :-1]).astype(np.float32)
    m[:, 3::2] = neq
    mt = np.zeros((B, S), dtype=np.float32)  # m~[s'] = m[162-s']
    sp = np.arange(2, S)
    mt[:, sp] = m[:, 162 - sp]
    t2 = np.float32(TAU * TAU)

    NIDX = 2 * NOCT * IDX3W + RIDXW
    # wrapped position of idx entry [p, f] within its 16-partition group
    p = np.arange(128)
    idx_all = []
    mask_all = []
    for k in range(NCORES):
        base = k * BPC
        idx = np.zeros((128, NIDX), dtype=np.uint16)
        for dirb in range(2):
            for o in range(NOCT):
                ib = (dirb * NOCT + o) * IDX3W
                for f in range(IDX3W):
                    pos = f * 16 + (p % 16)  # position in the 644-list
                    valid = pos < GW
                    u, s = pos // S, pos % S
                    g = p // 16
                    seq = base + 8 * o + g
                    if dirb == 0:
                        val = u[valid] * C + ext[seq[valid], s[valid]]
                    else:
                        # u-slot reversal + state reversal
                        val = (TSUB - 1 - u[valid]) * C + ext[
                            seq[valid], (S - 1) - s[valid]
                        ]
                    col = np.zeros(128, dtype=np.uint16)
                    col[valid] = val
                    idx[:, ib + f] = col
        jr = 2 * NOCT * IDX3W
        for f in range(RIDXW):
            pos = f * 16 + (p % 16)
            valid = pos < S
            col = np.zeros(128, dtype=np.uint16)
            col[valid] = (S - 1) - pos[valid]
            idx[:, jr + f] = col
        idx_all.append(idx)

        mask = np.zeros((128, S), dtype=np.float32)
        mask[0:64] = m[base : base + BPC] * t2
        mask[64:128] = mt[base : base + BPC] * t2
        mask_all.append(mask)
    return idx_all, mask_all


def gsel_host():
    g = np.zeros((128, 8), dtype=np.float32)
    for gg in range(8):
        g[16 * gg : 16 * gg + 16, gg] = 1.0
    return g


def kernel(y_true, y_pred):
    from concourse.bass_utils import run_bass_kernel_spmd

    y_pred = np.ascontiguousarray(np.asarray(y_pred), dtype=np.float32)
    idx_all, mask_all = _host_prep(y_true)

    nc = _build()
    in_maps = []
    for k in range(NCORES):
        b0 = k * BPC
        in_maps.append(
            {
                "y_pred": np.ascontiguousarray(y_pred[b0 : b0 + BPC]),
                "idx": idx_all[k],
                "mask": mask_all[k],
                "gsel": gsel_host(),
            }
        )
    res = run_bass_kernel_spmd(
        nc,
        in_maps,
        core_ids=list(range(NCORES)),
        trace=bool(int(os.environ.get("CTC_TRACE", "0"))),
    )
    out = np.concatenate([r["loss"] for r in res.results], axis=0)
    if res.exec_time_ns is not None:
        print(f"HW exec time: {res.exec_time_ns} ns", file=sys.stderr)
    return out.astype(np.float32)


# revision 19
# speedup vs baseline: 1.9263x; 1.6189x over previous
"""CTC loss (Keras ctc_batch_cost semantics) on 8 Trainium2 NeuronCores.

v6: forward+backward meet-in-the-middle, 84-wide bf16 label gathers.

Chain: each core handles 64 sequences; the DP state tile is [128, 163] bf16:
rows 0-63 run the forward alpha recurrence, rows 64-127 the backward gamma
recurrence in state-REVERSED order (both the same shifted form with the
tau^s tilt keeping their fp-range profiles overlapping at the join):

    Q'[r,s] = G_i[r,s] * (Q[r,s] + tau*Q[r,s-1] + Mt[r,s]*Q[r,s-2])

255 serial macro steps. Blank states (even s) all share one emission value
per (row, t), so the per-step update splits by parity into 5 DVE ops:

    x        = Q + tau*shift1(Q)              (scalar_tensor_tensor, full)
    y_o      = modd * Q[odd-2]                (dense x strided TT, 80)
    x[odd]  += y_o                            (strided TT, 80)
    Q'[odd]  = x[odd] * G_labels              (strided TT/stt, 80)
    Q'[even] = x[even] * G_blank_column       (tensor_scalar w/ AP scalars, 81)

and the gathers only fetch [blank, 80 labels] = 84-wide slots in bf16, which
cuts the SBUF->SBUF transpose-DMA traffic 4x vs full-width fp32 (that DMA
path sustains only ~15 GB/s and was the previous bottleneck). The fp32->bf16
cast rides the Z-sum activations for free (out tile repointed, +EPS folded).
Renormalization (every 8 steps, by row sum) runs inline on the DVE.

  loss[b] = sum_t ln Z[b,t] - sum_k ln z_f - sum_k ln z_b - ln P + 160 ln tau
"""

import functools
import os
import sys

import numpy as np

B, T, C, L = 512, 512, 128, 80
S = 2 * L + 1  # 161
BLANK = C - 1
EPS = 1e-7
NCORES = 8
BPC = B // NCORES  # 64 sequences per core
TC = 64  # time-chunk per macro chunk
NMC = 4  # macro chunks (255 chain steps)
TSUB = 4  # times per partition in the gather layout
NW = TC // TSUB  # 16 w-slots per sequence
NOCT = BPC // 8  # 8 seq-octets per core
RN = 8  # renormalize every 8 steps
NRENORM = 31
SPAD = S + 2
SW = 84  # gather slot: [blank, 80 labels, 3 pad]
GW = TSUB * SW  # 336
IDX3W = 22  # wrapped idx columns (336/16=21, padded even)
RIDXW = 12  # epilogue reversal idx columns
SG = S + 3
TAU = 0.3


def _emit_kernel(ctx, tc, ypred, idxt, maskt, moddt, losst):
    import concourse.bass as bass  # noqa: F401
    import concourse.mybir as mybir
    from concourse.ap import AP

    nc = tc.nc
    f32 = mybir.dt.float32
    bf16 = mybir.dt.bfloat16
    Alu = mybir.AluOpType
    Act = mybir.ActivationFunctionType

    singles = ctx.enter_context(tc.tile_pool(name="singles", bufs=1))
    ypool = ctx.enter_context(tc.tile_pool(name="ypool", bufs=2))
    gpool = ctx.enter_context(tc.tile_pool(name="gpool", bufs=2))
    g2pool = ctx.enter_context(tc.tile_pool(name="g2pool", bufs=6))
    small = ctx.enter_context(tc.tile_pool(name="small", bufs=2))
    finp = ctx.enter_context(tc.tile_pool(name="finp", bufs=8))
    epi = ctx.enter_context(tc.tile_pool(name="epi", bufs=1))
    psump = ctx.enter_context(tc.tile_pool(name="psum", bufs=2, space="PSUM"))

    # --- constants loaded once -------------------------------------------
    NIDX = 2 * NOCT * IDX3W + RIDXW
    idx_sb = singles.tile([128, NIDX], mybir.dt.uint16)
    nc.sync.dma_start(out=idx_sb[:, :], in_=idxt)
    m_sb = singles.tile([128, S], bf16)
    nc.sync.dma_start(out=m_sb[:, :], in_=maskt)
    modd_sb = singles.tile([128, 80], bf16)
    nc.sync.dma_start(out=modd_sb[:, :], in_=moddt)
    idx_scr = singles.tile([16, 1], mybir.dt.uint16)
    nc.gpsimd.tensor_copy(out=idx_scr[:, :], in_=idx_sb[0:16, 0:1])

    # Z accumulator: col = ((mc*2+dir)*NOCT + o)*TSUB + u
    zbig = singles.tile([128, 2 * NMC * NOCT * TSUB], f32)
    # group selector for the partition-axis W reduction via PE (host input)
    gsel = singles.tile([128, 8], f32)
    nc.sync.dma_start(out=gsel[:, :], in_=tc.gselt)

    # --- producers -------------------------------------------------------
    # Phase A: per-(chunk, direction, octet) fp32 load DMAs (2KB packets),
    # all triggered upfront; fwd on sync, bwd on scalar.
    ytbigs = {}
    for mc in range(NMC):
        for dirb in range(2):
            ybig = ypool.tile([128, NOCT, TSUB * C], f32, tag=f"ybig{dirb}")
            ytbigs[(mc, dirb)] = ybig
            for o in range(NOCT):
                if dirb == 0:
                    off = 8 * o * T * C + (mc * TC) * C
                    wstep = TSUB * C
                else:
                    # w-blocks time-reversed: partition (g,w) slot u holds
                    # y[seq, (448-64mc) + 4*(NW-1-w) + u]
                    off = (
                        8 * o * T * C
                        + ((7 - mc) * TC) * C
                        + (NW - 1) * TSUB * C
                    )
                    wstep = -TSUB * C
                src = AP(
                    ypred.tensor,
                    ypred.offset + off,
                    [
                        [T * C, 8],      # g (seq within octet)
                        [wstep, NW],     # w block (reversed for bwd)
                        [1, TSUB * C],   # (u, c) contiguous
                    ],
                )
                eng = nc.sync if dirb == 0 else nc.scalar
                eng.dma_start(out=ybig[:, o, :], in_=src)
    # Phase B: per chunk: Z-sum activations double as fp32->bf16+EPS casts,
    # then bf16 84-wide gathers and transpose DMAs.
    gtiles = []
    for mc in range(NMC):
        gtile = gpool.tile([128, TC * SW], bf16, tag="gchunk")
        for dirb in range(2):
            ybf = ypool.tile([128, NOCT, TSUB * C], bf16, tag=f"ybf{dirb}")
            for o in range(NOCT):
                yt = ytbigs[(mc, dirb)]
                for u in range(TSUB):
                    col = ((mc * 2 + dirb) * NOCT + o) * TSUB + u
                    nc.scalar.activation(
                        out=ybf[:, o, u * C : (u + 1) * C],
                        in_=yt[:, o, u * C : (u + 1) * C],
                        func=Act.Copy,
                        bias=EPS,
                        accum_out=zbig[:, col : col + 1],
                    )
                g2 = g2pool.tile([128, GW], bf16, tag=f"g2{dirb}")
                nc.gpsimd.tensor_copy(out=g2[0:16, 0:1], in_=ybf[0:16, o, 0:1])
                ib = (dirb * NOCT + o) * IDX3W
                nc.gpsimd.indirect_copy(
                    g2[:, :], ybf[:, o, :], idx_sb[:, ib : ib + IDX3W], True
                )
                # transpose: partition (g,w), free (u,slot) -> row
                # (8o+g+64*dir), free (4w+u)*SW + slot ; 128 x 672B packets
                r0 = 64 * dirb + 8 * o
                out_v = gtile[r0 : r0 + 8, :].rearrange(
                    "g (w q) -> g w q", w=NW
                )
                eng = nc.sync if dirb == 0 else nc.scalar
                eng.dma_start(out=out_v, in_=g2[:, :])
        gtiles.append(gtile)

    # --- the DP chain -----------------------------------------------------
    qa = singles.tile([128, SPAD], bf16)
    qb = singles.tile([128, SPAD], bf16)
    xt = singles.tile([128, S], bf16)
    yo = singles.tile([128, 80], bf16)
    zstash = singles.tile([128, NRENORM], f32)

    nc.vector.memset(qa[:, :], 0.0)
    nc.vector.memset(qb[:, 0:2], 0.0)
    # init: states 0,1 = gather slot cols 0 (blank), 1 (label 0)
    nc.vector.tensor_copy(out=qa[:, 2:3], in_=gtiles[0][:, 0:1])
    nc.vector.tensor_scalar(
        out=qa[:, 3:4], in0=gtiles[0][:, 1:2], scalar1=TAU, scalar2=0.0,
        op0=Alu.mult, op1=Alu.add,
    )

    bl_tiles = []
    for mc in range(NMC):
        blt = singles.tile([128, TC], f32)
        bl_tiles.append(blt)
    # blank cols of chunk 0 (strided bf16 -> fp32 copy; waits on the same
    # transposes the first chain step needs)
    nc.vector.tensor_copy(
        out=bl_tiles[0][:, :], in_=gtiles[0][:, 0 : TC * SW : SW]
    )

    rz_tiles = {}
    cur, nxt = qa, qb
    for i in range(1, 256):
        mc, toff = divmod(i, TC)
        if toff == 0 and mc > 0:
            nc.vector.tensor_copy(
                out=bl_tiles[mc][:, :], in_=gtiles[mc][:, 0 : TC * SW : SW]
            )
        g_lab = gtiles[mc][:, toff * SW + 1 : toff * SW + 81]
        g_bl = bl_tiles[mc][:, toff : toff + 1]
        # x[s] = q[s] + tau*q[s-1]   (cols: x[s] at col s)
        nc.vector.scalar_tensor_tensor(
            out=xt[:, :],
            in0=cur[:, 1 : S + 1],
            scalar=TAU,
            in1=cur[:, 2:SPAD],
            op0=Alu.mult,
            op1=Alu.add,
        )
        # y_o[k] = modd[k] * q[2k-1]  (odd target s=2k+1; q[s-2] at col s)
        nc.vector.tensor_tensor(
            out=yo[:, :],
            in0=modd_sb[:, :],
            in1=cur[:, 1 : S : 2],
            op=Alu.mult,
        )
        # x[odd] += y_o
        nc.vector.tensor_tensor(
            out=xt[:, 1 : S : 2], in0=xt[:, 1 : S : 2], in1=yo[:, :], op=Alu.add
        )
        if i % RN == 0 and (i // RN - 1) in rz_tiles:
            rz = rz_tiles.pop(i // RN - 1)
            nc.vector.scalar_tensor_tensor(
                out=nxt[:, 3 : SPAD : 2],
                in0=xt[:, 1 : S : 2],
                scalar=rz[:, :],
                in1=g_lab,
                op0=Alu.mult,
                op1=Alu.mult,
            )
            nc.vector.tensor_scalar(
                out=nxt[:, 2 : SPAD : 2],
                in0=xt[:, 0 : S : 2],
                scalar1=g_bl,
                scalar2=rz[:, :],
                op0=Alu.mult,
                op1=Alu.mult,
            )
        else:
            nc.vector.tensor_tensor(
                out=nxt[:, 3 : SPAD : 2], in0=xt[:, 1 : S : 2], in1=g_lab,
                op=Alu.mult,
            )
            nc.vector.tensor_scalar(
                out=nxt[:, 2 : SPAD : 2],
                in0=xt[:, 0 : S : 2],
                scalar1=g_bl,
                scalar2=1.0,
                op0=Alu.mult,
                op1=Alu.mult,
            )
        cur, nxt = nxt, cur
        if i % RN == RN - 1 and i < 255:
            k = i // RN
            nc.vector.reduce_sum(
                out=zstash[:, k : k + 1], in_=cur[:, 2:SPAD],
                axis=mybir.AxisListType.X,
            )
            rz = small.tile([128, 1], f32, tag="rz")
            nc.vector.reciprocal(out=rz[:, :], in_=zstash[:, k : k + 1])
            rz_tiles[k] = rz

    # --- epilogue ---------------------------------------------------------
    # beta' step on bottom rows: bt = g~ + tau*g~[-1] + m~t*g~[-2]
    bx = epi.tile([64, S], bf16, tag="bx")
    nc.vector.scalar_tensor_tensor(
        out=bx[:, :],
        in0=cur[64:128, 1 : S + 1],
        scalar=TAU,
        in1=cur[64:128, 2:SPAD],
        op0=Alu.mult,
        op1=Alu.add,
    )
    by = epi.tile([64, S], bf16, tag="by")
    nc.vector.tensor_tensor(
        out=by[:, :], in0=m_sb[64:128, :], in1=cur[64:128, 0:S], op=Alu.mult
    )
    btfull = singles.tile([128, SG], bf16)
    nc.vector.memset(btfull[:, :], 0.0)
    nc.vector.tensor_tensor(
        out=btfull[64:128, 0:S], in0=bx[:, :], in1=by[:, :], op=Alu.add
    )
    # state-reversal gather (group-shared reversing index list)
    btrev = singles.tile([128, SG], bf16)
    nc.gpsimd.tensor_copy(out=btrev[0:16, 0:1], in_=btfull[0:16, 0:1])
    jr = 2 * NOCT * IDX3W
    nc.gpsimd.indirect_copy(
        btrev[:, :], btfull[:, :], idx_sb[:, jr : jr + RIDXW], True
    )
    balign = epi.tile([64, S], bf16, tag="balign")
    nc.sync.dma_start(out=balign[:, :], in_=btrev[64:128, 0:S])
    # join dot: Phat[b] = sum_s alpha[b,s] * balign[b,s]
    pjunk = epi.tile([64, S], bf16, tag="pjunk")
    phat = finp.tile([64, 1], f32, tag="fin")
    nc.vector.scalar_tensor_tensor(
        out=pjunk[:, :],
        in0=cur[0:64, 2:SPAD],
        scalar=1.0,
        in1=balign[:, :],
        op0=Alu.mult,
        op1=Alu.mult,
        accum_out=phat[:, :],
    )
    lnp = finp.tile([64, 1], f32, tag="fin")
    nc.scalar.activation(out=lnp[:, :], in_=phat[:, :], func=Act.Ln)

    # renorm scale logs: R[r] = sum_k ln z[r,k]; per-seq Rsum = R[b] + R[64+b]
    lnzt = epi.tile([128, NRENORM], f32, tag="lnzt")
    nc.scalar.activation(out=lnzt[:, :], in_=zstash[:, :], func=Act.Ln)
    rfull = epi.tile([128, 1], f32, tag="rfull")
    nc.vector.reduce_sum(out=rfull[:, :], in_=lnzt[:, :], axis=mybir.AxisListType.X)
    rb = finp.tile([64, 1], f32, tag="fin")
    nc.sync.dma_start(out=rb[:, :], in_=rfull[64:128, :])
    rsum = finp.tile([64, 1], f32, tag="fin")
    nc.vector.tensor_tensor(
        out=rsum[:, :], in0=rfull[0:64, :], in1=rb[:, :], op=Alu.add
    )

    # softmax normalizer W: ln(zbig), reduce (u) then (mcd), PE group-sum
    NMCD = 2 * NMC
    lnZ = singles.tile([128, NMCD * NOCT * TSUB], f32)
    nc.scalar.activation(out=lnZ[:, :], in_=zbig[:, :], func=Act.Ln)
    wt1 = singles.tile([128, NMCD * NOCT], f32)
    lv = lnZ[:, :].rearrange("p (m o u) -> p (m o) u", m=NMCD, o=NOCT)
    nc.vector.reduce_sum(out=wt1[:, :], in_=lv, axis=mybir.AxisListType.X)
    wsum3 = singles.tile([128, NOCT], f32)
    lv2 = wt1[:, :].rearrange("p (m o) -> p o m", m=NMCD)
    nc.vector.reduce_sum(out=wsum3[:, :], in_=lv2, axis=mybir.AxisListType.X)
    psw = psump.tile([8, 8], f32, tag="ps1")
    nc.tensor.matmul(psw[:, :], lhsT=gsel[:, :], rhs=wsum3[:, :], start=True, stop=True)
    wsb = epi.tile([8, 8], f32, tag="wsb")
    nc.vector.tensor_copy(out=wsb[:, :], in_=psw[:, :])
    wb = finp.tile([BPC, 1], f32, tag="fin")
    for o in range(NOCT):
        nc.sync.dma_start(out=wb[8 * o : 8 * o + 8, :], in_=wsb[:, o : o + 1])

    # loss = W - Rsum - lnP + 160*ln(tau)
    t1 = finp.tile([BPC, 1], f32, tag="fin")
    nc.vector.tensor_tensor(out=t1[:, :], in0=wb[:, :], in1=rsum[:, :], op=Alu.subtract)
    t2 = finp.tile([BPC, 1], f32, tag="fin")
    nc.vector.tensor_tensor(out=t2[:, :], in0=t1[:, :], in1=lnp[:, :], op=Alu.subtract)
    lt = finp.tile([BPC, 1], f32, tag="fin")
    nc.vector.tensor_scalar(
        out=lt[:, :],
        in0=t2[:, :],
        scalar1=float(S - 1) * float(np.log(TAU)),
        scalar2=0.0,
        op0=Alu.add,
        op1=Alu.add,
    )
    nc.sync.dma_start(out=losst, in_=lt[:, :])


@functools.lru_cache(maxsize=4)
def _build():
    from contextlib import ExitStack

    import concourse.bacc as bacc
    import concourse.mybir as mybir
    import concourse.tile as tile

    nc = bacc.Bacc(trn_type="TRN2", target_bir_lowering=False)
    ypred = nc.dram_tensor("y_pred", [BPC, T, C], mybir.dt.float32, kind="ExternalInput")
    NIDX = 2 * NOCT * IDX3W + RIDXW
    idxt = nc.dram_tensor("idx", [128, NIDX], mybir.dt.uint16, kind="ExternalInput")
    maskt = nc.dram_tensor("mask", [128, S], mybir.dt.bfloat16, kind="ExternalInput")
    moddt = nc.dram_tensor("modd", [128, 80], mybir.dt.bfloat16, kind="ExternalInput")
    gselt = nc.dram_tensor("gsel", [128, 8], mybir.dt.float32, kind="ExternalInput")
    losst = nc.dram_tensor("loss", [BPC, 1], mybir.dt.float32, kind="ExternalOutput")
    with tile.TileContext(nc) as tc:
        tc.gselt = gselt[:, :]
        with ExitStack() as ctx:
            _emit_kernel(
                ctx, tc, ypred[:, :, :], idxt[:, :], maskt[:, :], moddt[:, :],
                losst[:, :],
            )
    nc.compile()
    return nc


def _host_prep(y_true):
    """84-wide octet gather indices (fwd, bwd-reversed, epilogue-reversal)
    and tau^2-scaled masks (full for the epilogue, odd-dense for the chain)."""
    import ml_dtypes

    bf = ml_dtypes.bfloat16
    y_true = np.asarray(y_true).astype(np.int64)
    lab = y_true
    ext = np.full((B, S), BLANK, dtype=np.int64)
    ext[:, 1::2] = lab
    m = np.zeros((B, S), dtype=np.float32)
    m[:, 1] = 1.0
    neq = (lab[:, 1:] != lab[:, :-1]).astype(np.float32)
    m[:, 3::2] = neq
    mt = np.zeros((B, S), dtype=np.float32)  # m~[s'] = m[162-s']
    sp = np.arange(2, S)
    mt[:, sp] = m[:, 162 - sp]
    t2 = np.float32(TAU * TAU)

    NIDX = 2 * NOCT * IDX3W + RIDXW
    p = np.arange(128)
    idx_all, mask_all, modd_all = [], [], []
    for k in range(NCORES):
        base = k * BPC
        idx = np.zeros((128, NIDX), dtype=np.uint16)
        for dirb in range(2):
            for o in range(NOCT):
                ib = (dirb * NOCT + o) * IDX3W
                for f in range(IDX3W):
                    pos = f * 16 + (p % 16)  # position in the 336-list
                    valid = pos < GW
                    u, slot = pos // SW, pos % SW
                    g = p // 16
                    seq = base + 8 * o + g
                    val = np.zeros(128, dtype=np.uint16)
                    lab_ok = valid & (slot >= 1) & (slot <= 80)
                    bl_ok = valid & (slot == 0)
                    if dirb == 0:
                        val[bl_ok] = u[bl_ok] * C + BLANK
                        val[lab_ok] = u[lab_ok] * C + lab[
                            seq[lab_ok], slot[lab_ok] - 1
                        ]
                    else:
                        ur = TSUB - 1 - u
                        val[bl_ok] = ur[bl_ok] * C + BLANK
                        val[lab_ok] = ur[lab_ok] * C + lab[
                            seq[lab_ok], 80 - slot[lab_ok]
                        ]
                    idx[:, ib + f] = val
        jr = 2 * NOCT * IDX3W
        for f in range(RIDXW):
            pos = f * 16 + (p % 16)
            valid = pos < S
            col = np.zeros(128, dtype=np.uint16)
            col[valid] = (S - 1) - pos[valid]
            idx[:, jr + f] = col
        idx_all.append(idx)

        mask = np.zeros((128, S), dtype=np.float32)
        mask[0:64] = m[base : base + BPC] * t2
        mask[64:128] = mt[base : base + BPC] * t2
        mask_all.append(mask.astype(bf))
        # dense odd-target mask: modd[r, k] = mask[r, 2k+1]
        modd_all.append(np.ascontiguousarray(mask[:, 1::2]).astype(bf))
    return idx_all, mask_all, modd_all


def gsel_host():
    g = np.zeros((128, 8), dtype=np.float32)
    for gg in range(8):
        g[16 * gg : 16 * gg + 16, gg] = 1.0
    return g


def kernel(y_true, y_pred):
    from concourse.bass_utils import run_bass_kernel_spmd

    y_pred = np.ascontiguousarray(np.asarray(y_pred), dtype=np.float32)
    idx_all, mask_all, modd_all = _host_prep(y_true)

    nc = _build()
    in_maps = []
    for k in range(NCORES):
        b0 = k * BPC
        in_maps.append(
            {
                "y_pred": np.ascontiguousarray(y_pred[b0 : b0 + BPC]),
                "idx": idx_all[k],
                "mask": mask_all[k],
                "modd": modd_all[k],
                "gsel": gsel_host(),
            }
        )
    res = run_bass_kernel_spmd(
        nc,
        in_maps,
        core_ids=list(range(NCORES)),
        trace=bool(int(os.environ.get("CTC_TRACE", "0"))),
    )
    out = np.concatenate([r["loss"] for r in res.results], axis=0)
    if res.exec_time_ns is not None:
        print(f"HW exec time: {res.exec_time_ns} ns", file=sys.stderr)
    return out.astype(np.float32)
